# revision 13
# baseline (speedup 1.0000x reference)
"""Mixture-of-Experts Trainium2 kernel (8-core SPMD, token-sharded).

Reference: coarse top-K expert selection from the gate applied to the global
token sum, then dense K-expert FFN over all tokens with per-token softmax
gating over the K selected experts.

Hybrid-precision device strategy ("E-split"). The pointwise identity
    gelu(v) = v/2 + E(v),   E even, small (std ~0.1)
splits each expert's FFN into an exact linear path and a residual path:
    gelu(x@W1 + b1) @ W2 = x @ (W1@W2)/2 + (b1/2 + E(v)) @ W2
The linear path is a cheap [D,D] bf16 matmul (precomputed W1@W2 on host).
Only the residual E - mu (mean-removed, quantized fp8e4) goes through the
big [DF,D] contraction, which then runs as fp8 DoubleRow matmuls at ~1.8x
the bf16 rate.  L1 (x@W1, needed to evaluate E) stays bf16 for accuracy.
The exact rank-1 term sum_k gw_k * (mu_k*colsum(W2_k) + b2_k) is added on
host (gw recomputed exactly there; mismatch vs device gw is ~1e-5).

Scales: w1 x32 (psum=32u, gelu scale 1/32), E x4, w2 x32, lin M = 64*W1W2
=> combine psum = 128 * eo; softmax gating weights pre-divided by 128.

Per core (T=2048 tokens): gating bf16 (token-major), then per expert:
L1 64 psum tiles (4 bf16 MMs each) -> gelu (ACT) -> E fp8 (2 DVE ops),
combine 16 psum tiles (4 bf16 lin MMs + 8 fp8-DR MMs) -> acc (+)= po*gw.
Output stores stream per-tile during the last expert's combine.
"""

import numpy as np
import ml_dtypes
from contextlib import ExitStack

import bass_rust as _bass_rust
import concourse.bass as bass
import concourse.mybir as mybir
import concourse.tile as tile
from concourse.bass_utils import run_bass_kernel_spmd

BF16 = mybir.dt.bfloat16
FP8 = mybir.dt.float8e4
F32 = mybir.dt.float32
N_CORES = 8
P = 128

S_W1 = 0.5     # w1 pre-scale (L1 psum = u/2, so E = g - psum in ONE DVE op)
S_W2 = 128.0   # w2 pre-scale
S_PS = S_W2                # combine psum scale (=128)
S_M = S_PS / 2.0           # lin path: M = 64 * (W1 @ W2)


# ---------------------------------------------------------------------------
# Workaround for walrus "Too many sync wait commands": this walrus build
# accepts at most one semaphore wait in a single instruction's sync_info,
# but Tile's scheduler (and its kernel-tail drain) can attach several.
# Post-pass: move excess waits onto standalone EventSemaphore instructions
# inserted immediately before the offender on the same engine.
# ---------------------------------------------------------------------------
_split_ctr = [0]


def _split_multi_waits(nc):
    for f in nc.m.functions:
        for blk in f.blocks:
            insts = blk.instructions
            i = 0
            while i < len(insts):
                inst = insts[i]
                si = getattr(inst, "sync_info", None)
                waits = list(si.on_wait) if si is not None and si.on_wait else []
                if len(waits) > 1:
                    si.on_wait = waits[-1:]
                    for w in waits[:-1]:
                        _split_ctr[0] += 1
                        ev = mybir.InstEventSemaphore(
                            name=f"I-wsplit-{_split_ctr[0]}", ins=[], outs=[]
                        )
                        ev.engine = inst.engine
                        ev.sync_info = _bass_rust.SyncInfo(
                            on_wait=[w], on_update=[]
                        )
                        insts.insert(i, ev)
                        i += 1
                i += 1


# ---------------------------------------------------------------------------
# Device kernel
# ---------------------------------------------------------------------------
def build_moe_kernel(K: int, T: int, D: int, DF: int):
    assert T % 512 == 0 and D % P == 0 and DF % 256 == 0
    TT = T // P       # 128-token tiles
    TC = T // 512     # 512-token chunks
    DC = D // P       # D chunks of 128
    FC = DF // P      # F chunks of 128
    C8 = DF // 256    # DoubleRow chunks (256-deep each)

    nc = bass.Bass("TRN2", target_bir_lowering=False)

    xT = nc.declare_dram_parameter("xT", [D, T], BF16, isOutput=False)
    w1b = nc.declare_dram_parameter("w1b", [K, D, DF], BF16, isOutput=False)
    w2f = nc.declare_dram_parameter("w2f", [K, P, FC, D], FP8, isOutput=False)
    mB = nc.declare_dram_parameter("mB", [K, D, D], BF16, isOutput=False)
    gwsT = nc.declare_dram_parameter("gwsT", [P, DC * K], BF16, isOutput=False)
    gbb = nc.declare_dram_parameter("gbb", [P, K], F32, isOutput=False)
    b1p = nc.declare_dram_parameter("b1p", [K, P, FC], F32, isOutput=False)
    nmu = nc.declare_dram_parameter("nmu", [P, K], F32, isOutput=False)
    out = nc.declare_dram_parameter("out", [T, D], F32, isOutput=True)

    mult = mybir.AluOpType.mult
    add = mybir.AluOpType.add
    gelu_fn = mybir.ActivationFunctionType.Gelu_apprx_tanh
    exp_fn = mybir.ActivationFunctionType.Exp
    DR = mybir.MatmulPerfMode.DoubleRow

    with tile.TileContext(nc) as tc:
        with ExitStack() as ctx:
            persist = ctx.enter_context(tc.tile_pool(name="persist", bufs=1))
            w1p = ctx.enter_context(tc.tile_pool(name="w1p", bufs=2 * DC))
            w2p = ctx.enter_context(tc.tile_pool(name="w2p", bufs=2))
            ep = ctx.enter_context(tc.tile_pool(name="ep", bufs=C8))
            gp = ctx.enter_context(tc.tile_pool(name="gp", bufs=3))
            sm = ctx.enter_context(tc.tile_pool(name="sm", bufs=4))
            psA = ctx.enter_context(tc.tile_pool(name="psA", bufs=3, space="PSUM"))
            psB = ctx.enter_context(tc.tile_pool(name="psB", bufs=5, space="PSUM"))

            # ---- persistent loads. Emission order = DMA queue order:
            # tiny tensors first (gating needs gws immediately), then x
            # chunks interleaved with expert-0 w1 chunks so gating can start
            # on x chunk 0 while w1 still streams. ----
            gws_sb = persist.tile([P, DC * K], BF16, tag="gws", name="gws_sb")
            nc.sync.dma_start(gws_sb[:], gwsT[:])
            gbb_sb = persist.tile([P, K], F32, tag="gbb", name="gbb_sb")
            nc.sync.dma_start(gbb_sb[:], gbb[:])
            nmu_sb = persist.tile([P, K], F32, tag="nmu", name="nmu_sb")
            nc.sync.dma_start(nmu_sb[:], nmu[:])
            b1_sb = []
            for k in range(K):
                t = persist.tile([P, FC], F32, tag=f"b1_{k}", name=f"b1_{k}")
                nc.sync.dma_start(t[:], b1p[k])
                b1_sb.append(t)

            xt = [
                persist.tile([P, T], BF16, tag=f"xt{dc}", name=f"xt{dc}")
                for dc in range(DC)
            ]
            w1t0 = [
                w1p.tile([P, DF], BF16, tag="w1", name=f"w1_0_{dc}")
                for dc in range(DC)
            ]
            # token-chunk-major x transfer: gating round r needs only token
            # chunk r across all 4 d-chunks (0.5MB), so matmuls start ~6us
            # earlier than with whole-chunk transfers
            for tcc in range(TC):
                for dc in range(DC):
                    nc.sync.dma_start(
                        xt[dc][:, tcc * 512:(tcc + 1) * 512],
                        xT[dc * P:(dc + 1) * P, tcc * 512:(tcc + 1) * 512],
                    )
            for dc in range(DC):
                nc.sync.dma_start(w1t0[dc][:], w1b[0, dc * P:(dc + 1) * P, :])
            w2t0 = w2p.tile([P, FC, D], FP8, tag="w2", name="w2_0")
            nc.sync.dma_start(w2t0[:], w2f[0])

            # lin-path moving matrices, all experts resident
            m_sb = []
            for k in range(K):
                t = persist.tile([P, DC, D], BF16, tag=f"m{k}", name=f"m{k}")
                for dc in range(DC):
                    nc.sync.dma_start(
                        t[:, dc:dc + 1, :], mB[k, dc * P:(dc + 1) * P, :]
                    )
                m_sb.append(t)

            acc = [
                persist.tile([P, D], F32, tag=f"acc{t}", name=f"acc{t}")
                for t in range(TT)
            ]
            gw_sb = [
                persist.tile([P, K], F32, tag=f"gw{t}", name=f"gw{t}")
                for t in range(TT)
            ]

            # ---- gating softmax (token-major); gw_sb holds softmax/S_PS ----
            # dc-outer rounds of 4 token tiles: the first matmuls only need
            # x chunk 0, so gating starts while the rest of x still streams.
            RT = 4
            for rnd in range(0, TT, RT):
                tts = range(rnd, min(rnd + RT, TT))
                pls = {}
                for tt in tts:
                    pls[tt] = psB.tile([P, 512], F32, tag="po", name=f"pl{tt}")
                for dc in range(DC):
                    for tt in tts:
                        nc.tensor.matmul(
                            pls[tt][:, 0:K],
                            xt[dc][:, tt * P:(tt + 1) * P],
                            gws_sb[:, dc * K:(dc + 1) * K],
                            start=(dc == 0),
                            stop=(dc == DC - 1),
                        )
                for tt in tts:
                    pl = pls[tt]
                    l_sb = sm.tile([P, K], F32, tag="l", name=f"l{tt}")
                    nc.vector.tensor_add(l_sb[:], pl[:, 0:K], gbb_sb[:])
                    negmax = sm.tile([P, 1], F32, tag="negmax",
                                     name=f"negmax{tt}")
                    nc.vector.reduce_max(
                        negmax[:], l_sb[:], axis=mybir.AxisListType.X,
                        negate=True,
                    )
                    z = sm.tile([P, 1], F32, tag="z", name=f"z{tt}")
                    nc.scalar.activation(
                        gw_sb[tt][:], l_sb[:], exp_fn,
                        bias=negmax[:, 0:1], accum_out=z[:, 0:1],
                    )
                    rz = sm.tile([P, 1], F32, tag="rz", name=f"rz{tt}")
                    nc.vector.reciprocal(rz[:], z[:, 0:1])
                    nc.vector.tensor_scalar(
                        gw_sb[tt][:], gw_sb[tt][:], rz[:, 0:1], 1.0 / S_PS,
                        op0=mult, op1=mult,
                    )

            # ---- experts ----
            for k in range(K):
                if k == 0:
                    w1t, w2t8 = w1t0, w2t0
                else:
                    w1t = []
                    for dc in range(DC):
                        t = w1p.tile([P, DF], BF16, tag="w1", name=f"w1_{k}_{dc}")
                        nc.sync.dma_start(t[:], w1b[k, dc * P:(dc + 1) * P, :])
                        w1t.append(t)
                    w2t8 = w2p.tile([P, FC, D], FP8, tag="w2", name=f"w2_{k}")
                    nc.sync.dma_start(w2t8[:], w2f[k])

                # L1 (bf16): psum = u/2  [f-major], then
                #   g = gelu(2*psum + b1)            (ACT, bf16)
                #   E8 = (g + (-mu)) - psum           (DVE stt, fp8) = E - mu
                ec = [
                    ep.tile([P, 2, T], FP8, tag="E", name=f"E_{k}_{c}")
                    for c in range(C8)
                ]
                for fc in range(FC):
                    for tcc in range(TC):
                        ph = psA.tile([P, 512], F32, tag="ph",
                                      name=f"ph_{k}_{fc}_{tcc}")
                        for dc in range(DC):
                            nc.tensor.matmul(
                                ph[:],
                                w1t[dc][:, fc * P:(fc + 1) * P],
                                xt[dc][:, tcc * 512:(tcc + 1) * 512],
                                start=(dc == 0),
                                stop=(dc == DC - 1),
                            )
                        g = gp.tile([P, 512], BF16, tag="g",
                                    name=f"g_{k}_{fc}_{tcc}")
                        nc.scalar.activation(
                            g[:], ph[:], gelu_fn,
                            bias=b1_sb[k][:, fc:fc + 1], scale=1.0 / S_W1,
                        )
                        nc.vector.scalar_tensor_tensor(
                            ec[fc // 2][:, (fc % 2):(fc % 2) + 1,
                                        tcc * 512:(tcc + 1) * 512],
                            g[:], nmu_sb[:, k:k + 1], ph[:],
                            op0=add, op1=mybir.AluOpType.subtract,
                        )

                # combine: po = x@M (bf16) + E8 @ w2 (fp8 DoubleRow)
                #          acc (+)= po * gw
                for tt in range(TT):
                    po = psB.tile([P, 512], F32, tag="po", name=f"po_{k}_{tt}")
                    for dc in range(DC):
                        nc.tensor.matmul(
                            po[:, 0:D],
                            xt[dc][:, tt * P:(tt + 1) * P],
                            m_sb[k][:, dc:dc + 1, :],
                            start=(dc == 0),
                            stop=False,
                        )
                    for c in range(C8):
                        nc.tensor.matmul(
                            po[:, 0:D],
                            ec[c][:, :, tt * P:(tt + 1) * P],
                            w2t8[:, 2 * c:2 * c + 2, :],
                            start=False,
                            stop=(c == C8 - 1),
                            perf_mode=DR,
                        )
                    if k == 0:
                        nc.vector.tensor_scalar_mul(
                            acc[tt][:], po[:, 0:D], gw_sb[tt][:, 0:1]
                        )
                    else:
                        nc.vector.scalar_tensor_tensor(
                            acc[tt][:], po[:, 0:D], gw_sb[tt][:, k:k + 1],
                            acc[tt][:], op0=mult, op1=add,
                        )
                    if k == K - 1:
                        nc.sync.dma_start(
                            out[tt * P:(tt + 1) * P, :], acc[tt][:]
                        )

    _split_multi_waits(nc)
    return nc


# ---------------------------------------------------------------------------
# Host wrapper
# ---------------------------------------------------------------------------
_NC_CACHE: dict = {}


def _get_nc(K: int, T: int, D: int, DF: int):
    key = (K, T, D, DF)
    if key not in _NC_CACHE:
        _NC_CACHE[key] = build_moe_kernel(K, T, D, DF)
    return _NC_CACHE[key]


def _softmax(x, axis=-1):
    m = np.max(x, axis=axis, keepdims=True)
    e = np.exp(x - m)
    return e / np.sum(e, axis=axis, keepdims=True)


def _gelu_tanh_mean(sigma, b1):
    """E_z[gelu_tanh(b1 + sigma*z)], z~N(0,1), vectorized over features."""
    zg = np.linspace(-8.0, 8.0, 401)
    wg = np.exp(-0.5 * zg * zg)
    wg /= wg.sum()
    v = b1[:, None] + sigma[:, None] * zg[None, :]       # [DF, NZ]
    g = 0.5 * v * (1.0 + np.tanh(np.sqrt(2.0 / np.pi) * (v + 0.044715 * v**3)))
    return (g * wg[None, :]).sum(axis=1)                 # [DF]


def _f8(a):
    return np.clip(np.asarray(a, np.float32), -240.0, 240.0).astype(
        ml_dtypes.float8_e4m3fn
    )


def run(inputs: dict, trace: bool = False, tmpdir: str | None = None):
    x = np.asarray(inputs["x"], dtype=np.float32)
    gate_w = np.asarray(inputs["gate_w"], dtype=np.float32)
    gate_b = np.asarray(inputs["gate_b"], dtype=np.float32)
    w1 = np.asarray(inputs["w1"], dtype=np.float32)
    b1 = np.asarray(inputs["b1"], dtype=np.float32)
    w2 = np.asarray(inputs["w2"], dtype=np.float32)
    b2 = np.asarray(inputs["b2"], dtype=np.float32)
    K = int(inputs["num_available"])

    B, S, D = x.shape
    DF = w1.shape[2]
    Ttot = B * S
    T = Ttot // N_CORES
    DC = D // P
    FC = DF // P

    # Coarse routing on host (tiny): gate applied to the global token sum.
    ksum = x.sum(axis=(0, 1))
    coarse = gate_w @ ksum + gate_b
    idx = np.argsort(-coarse, kind="stable")[:K]

    gws, gbs = gate_w[idx], gate_b[idx]
    w1s = np.ascontiguousarray(w1[idx])                  # [K,D,DF] f32
    b1s = np.ascontiguousarray(b1[idx], dtype=np.float32)
    w2s = np.ascontiguousarray(w2[idx])                  # [K,DF,D] f32
    b2s = np.ascontiguousarray(b2[idx], dtype=np.float32)

    # device tensors
    w1b = (S_W1 * w1s).astype(ml_dtypes.bfloat16)        # [K,D,DF]
    w2f = np.ascontiguousarray(
        _f8(S_W2 * w2s).reshape(K, FC, P, D).transpose(0, 2, 1, 3)
    )                                                    # [K,P,FC,D] fp8
    mB = (S_M * np.matmul(w1s, w2s)).astype(ml_dtypes.bfloat16)  # [K,D,D]

    # E-residual means (exact Gaussian statistics: u_f ~ N(0, ||w1[:,f]||)).
    mus = np.empty(K, np.float32)
    for k in range(K):
        sigma = np.linalg.norm(w1s[k], axis=0)           # [DF]
        mus[k] = _gelu_tanh_mean(sigma, b1s[k]).mean() - 0.0
    nmu = np.ascontiguousarray(
        np.broadcast_to(-mus[None, :], (P, K)), dtype=np.float32
    )

    gwsT = np.ascontiguousarray(
        gws.T.reshape(DC, P, K).transpose(1, 0, 2).reshape(P, DC * K)
    ).astype(ml_dtypes.bfloat16)
    gbb = np.ascontiguousarray(np.broadcast_to(gbs[None, :], (P, K)),
                               dtype=np.float32)
    b1p = np.ascontiguousarray(
        b1s.reshape(K, FC, P).transpose(0, 2, 1), dtype=np.float32
    )

    xf = x.reshape(Ttot, D)
    xT_bf = np.ascontiguousarray(xf.T).astype(ml_dtypes.bfloat16)

    nc = _get_nc(K, T, D, DF)
    in_maps = []
    for c in range(N_CORES):
        in_maps.append({
            "xT": np.ascontiguousarray(xT_bf[:, c * T:(c + 1) * T]),
            "w1b": w1b,
            "w2f": w2f,
            "mB": mB,
            "gwsT": gwsT,
            "gbb": gbb,
            "b1p": b1p,
            "nmu": nmu,
        })

    res = run_bass_kernel_spmd(
        nc, in_maps, list(range(N_CORES)), trace=trace, tmpdir=tmpdir
    )
    outp = np.concatenate(
        [res.results[c]["out"] for c in range(N_CORES)], axis=0
    ).reshape(B, S, D).astype(np.float32)

    # Exact rank-1 correction: sum_k gw_k * (mu_k*colsum(W2_k) + b2_k).
    # gw recomputed on host in fp32; mismatch vs device bf16 gw is O(1e-5).
    C = mus[:, None] * w2s.sum(axis=1) + b2s             # [K, D]
    logits = xf @ gws.T + gbs[None, :]
    gwh = _softmax(logits, axis=1).astype(np.float32)
    outp = outp + (gwh @ C).reshape(B, S, D)

    return outp, res


def kernel(**inputs) -> np.ndarray:
    outp, _ = run(inputs, trace=False)
    return outp


# revision 14
# speedup vs baseline: 1.2014x; 1.2014x over previous
"""Mixture-of-Experts Trainium2 kernel (8-core SPMD, token-sharded).

Reference: coarse top-K expert selection from the gate applied to the global
token sum, then dense K-expert FFN over all tokens with per-token softmax
gating over the K selected experts.

Hybrid-precision device strategy ("E-split"). The pointwise identity
    gelu(v) = v/2 + E(v),   E even, small (std ~0.1)
splits each expert's FFN into an exact linear path and a residual path:
    gelu(x@W1 + b1) @ W2 = x @ (W1@W2)/2 + (b1/2 + E(v)) @ W2
The linear path is a cheap [D,D] bf16 matmul (precomputed W1@W2 on host).
Only the residual E - mu (mean-removed, quantized fp8e4) goes through the
big [DF,D] contraction, which then runs as fp8 DoubleRow matmuls at ~1.8x
the bf16 rate.  L1 (x@W1, needed to evaluate E) stays bf16 for accuracy.
The exact rank-1 term sum_k gw_k * (mu_k*colsum(W2_k) + b2_k) is added on
host (gw recomputed exactly there; mismatch vs device gw is ~1e-5).

Scales: w1 x32 (psum=32u, gelu scale 1/32), E x4, w2 x32, lin M = 64*W1W2
=> combine psum = 128 * eo; softmax gating weights pre-divided by 128.

Per core (T=2048 tokens): gating bf16 (token-major), then per expert:
L1 64 psum tiles (4 bf16 MMs each) -> gelu (ACT) -> E fp8 (2 DVE ops),
combine 16 psum tiles (4 bf16 lin MMs + 8 fp8-DR MMs) -> acc (+)= po*gw.
Output stores stream per-tile during the last expert's combine.
"""

import numpy as np
import ml_dtypes
from contextlib import ExitStack

import bass_rust as _bass_rust
import concourse.bass as bass
import concourse.mybir as mybir
import concourse.tile as tile
from concourse.bass_utils import run_bass_kernel_spmd

BF16 = mybir.dt.bfloat16
FP8 = mybir.dt.float8e4
F32 = mybir.dt.float32
N_CORES = 8
P = 128

S_W1 = 0.5     # w1 pre-scale (L1 psum = u/2, so E = g - psum in ONE DVE op)
S_W2 = 128.0   # w2 pre-scale
S_PS = S_W2                # combine psum scale (=128)
S_M = S_PS / 2.0           # lin path: M = 64 * (W1 @ W2)


# ---------------------------------------------------------------------------
# Workaround for walrus "Too many sync wait commands": this walrus build
# accepts at most one semaphore wait in a single instruction's sync_info,
# but Tile's scheduler (and its kernel-tail drain) can attach several.
# Post-pass: move excess waits onto standalone EventSemaphore instructions
# inserted immediately before the offender on the same engine.
# ---------------------------------------------------------------------------
_split_ctr = [0]


def _split_multi_waits(nc):
    for f in nc.m.functions:
        for blk in f.blocks:
            insts = blk.instructions
            i = 0
            while i < len(insts):
                inst = insts[i]
                si = getattr(inst, "sync_info", None)
                waits = list(si.on_wait) if si is not None and si.on_wait else []
                if len(waits) > 1:
                    si.on_wait = waits[-1:]
                    for w in waits[:-1]:
                        _split_ctr[0] += 1
                        ev = mybir.InstEventSemaphore(
                            name=f"I-wsplit-{_split_ctr[0]}", ins=[], outs=[]
                        )
                        ev.engine = inst.engine
                        ev.sync_info = _bass_rust.SyncInfo(
                            on_wait=[w], on_update=[]
                        )
                        insts.insert(i, ev)
                        i += 1
                i += 1


# ---------------------------------------------------------------------------
# Device kernel
# ---------------------------------------------------------------------------
def build_moe_kernel(K: int, T: int, D: int, DF: int):
    assert T % 512 == 0 and D % P == 0 and DF % 256 == 0
    TT = T // P       # 128-token tiles
    TC = T // 512     # 512-token chunks
    DC = D // P       # D chunks of 128
    FC = DF // P      # F chunks of 128
    C8 = DF // 256    # DoubleRow chunks (256-deep each)

    nc = bass.Bass("TRN2", target_bir_lowering=False)

    xT = nc.declare_dram_parameter("xT", [D, T], BF16, isOutput=False)
    w1b = nc.declare_dram_parameter("w1b", [K, D, DF], BF16, isOutput=False)
    w2f = nc.declare_dram_parameter("w2f", [K, P, FC, D], FP8, isOutput=False)
    mB = nc.declare_dram_parameter("mB", [K, D, D], BF16, isOutput=False)
    gwsT = nc.declare_dram_parameter("gwsT", [P, DC * K], BF16, isOutput=False)
    gbb = nc.declare_dram_parameter("gbb", [P, K], F32, isOutput=False)
    b1p = nc.declare_dram_parameter("b1p", [K, P, FC], F32, isOutput=False)
    nmu = nc.declare_dram_parameter("nmu", [P, K], F32, isOutput=False)
    out = nc.declare_dram_parameter("out", [T, D], F32, isOutput=True)

    mult = mybir.AluOpType.mult
    add = mybir.AluOpType.add
    gelu_fn = mybir.ActivationFunctionType.Gelu_apprx_tanh
    exp_fn = mybir.ActivationFunctionType.Exp
    DR = mybir.MatmulPerfMode.DoubleRow

    with tile.TileContext(nc) as tc:
        with ExitStack() as ctx:
            persist = ctx.enter_context(tc.tile_pool(name="persist", bufs=1))
            w1p = ctx.enter_context(tc.tile_pool(name="w1p", bufs=2 * DC))
            w2p = ctx.enter_context(tc.tile_pool(name="w2p", bufs=2))
            ep = ctx.enter_context(tc.tile_pool(name="ep", bufs=C8))
            gp = ctx.enter_context(tc.tile_pool(name="gp", bufs=3))
            sm = ctx.enter_context(tc.tile_pool(name="sm", bufs=4))
            psA = ctx.enter_context(tc.tile_pool(name="psA", bufs=3, space="PSUM"))
            psB = ctx.enter_context(tc.tile_pool(name="psB", bufs=5, space="PSUM"))

            # ---- persistent loads. Emission order = DMA queue order:
            # tiny tensors first (gating needs gws immediately), then x
            # chunks interleaved with expert-0 w1 chunks so gating can start
            # on x chunk 0 while w1 still streams. ----
            gws_sb = persist.tile([P, DC * K], BF16, tag="gws", name="gws_sb")
            nc.sync.dma_start(gws_sb[:], gwsT[:])
            gbb_sb = persist.tile([P, K], F32, tag="gbb", name="gbb_sb")
            nc.sync.dma_start(gbb_sb[:], gbb[:])
            nmu_sb = persist.tile([P, K], F32, tag="nmu", name="nmu_sb")
            nc.sync.dma_start(nmu_sb[:], nmu[:])
            b1_sb = []
            for k in range(K):
                t = persist.tile([P, FC], F32, tag=f"b1_{k}", name=f"b1_{k}")
                nc.sync.dma_start(t[:], b1p[k])
                b1_sb.append(t)

            xt = [
                persist.tile([P, T], BF16, tag=f"xt{dc}", name=f"xt{dc}")
                for dc in range(DC)
            ]
            w1t0 = [
                w1p.tile([P, DF], BF16, tag="w1", name=f"w1_0_{dc}")
                for dc in range(DC)
            ]
            for dc in range(DC):
                nc.sync.dma_start(xt[dc][:], xT[dc * P:(dc + 1) * P, :])
            for dc in range(DC):
                nc.sync.dma_start(w1t0[dc][:], w1b[0, dc * P:(dc + 1) * P, :])
            w2t0 = w2p.tile([P, FC, D], FP8, tag="w2", name="w2_0")
            nc.sync.dma_start(w2t0[:], w2f[0])

            # lin-path moving matrices, all experts resident
            m_sb = []
            for k in range(K):
                t = persist.tile([P, DC, D], BF16, tag=f"m{k}", name=f"m{k}")
                for dc in range(DC):
                    nc.sync.dma_start(
                        t[:, dc:dc + 1, :], mB[k, dc * P:(dc + 1) * P, :]
                    )
                m_sb.append(t)

            acc = [
                persist.tile([P, D], F32, tag=f"acc{t}", name=f"acc{t}")
                for t in range(TT)
            ]
            gw_sb = [
                persist.tile([P, K], F32, tag=f"gw{t}", name=f"gw{t}")
                for t in range(TT)
            ]

            # ---- gating softmax (token-major); gw_sb holds softmax/S_PS ----
            # dc-outer rounds of 4 token tiles: the first matmuls only need
            # x chunk 0, so gating starts while the rest of x still streams.
            RT = 4
            for rnd in range(0, TT, RT):
                tts = range(rnd, min(rnd + RT, TT))
                pls = {}
                for tt in tts:
                    pls[tt] = psB.tile([P, 512], F32, tag="po", name=f"pl{tt}")
                for dc in range(DC):
                    for tt in tts:
                        nc.tensor.matmul(
                            pls[tt][:, 0:K],
                            xt[dc][:, tt * P:(tt + 1) * P],
                            gws_sb[:, dc * K:(dc + 1) * K],
                            start=(dc == 0),
                            stop=(dc == DC - 1),
                        )
                for tt in tts:
                    pl = pls[tt]
                    l_sb = sm.tile([P, K], F32, tag="l", name=f"l{tt}")
                    nc.vector.tensor_add(l_sb[:], pl[:, 0:K], gbb_sb[:])
                    negmax = sm.tile([P, 1], F32, tag="negmax",
                                     name=f"negmax{tt}")
                    nc.vector.reduce_max(
                        negmax[:], l_sb[:], axis=mybir.AxisListType.X,
                        negate=True,
                    )
                    z = sm.tile([P, 1], F32, tag="z", name=f"z{tt}")
                    nc.scalar.activation(
                        gw_sb[tt][:], l_sb[:], exp_fn,
                        bias=negmax[:, 0:1], accum_out=z[:, 0:1],
                    )
                    rz = sm.tile([P, 1], F32, tag="rz", name=f"rz{tt}")
                    nc.vector.reciprocal(rz[:], z[:, 0:1])
                    nc.vector.tensor_scalar(
                        gw_sb[tt][:], gw_sb[tt][:], rz[:, 0:1], 1.0 / S_PS,
                        op0=mult, op1=mult,
                    )

            # ---- experts ----
            for k in range(K):
                if k == 0:
                    w1t, w2t8 = w1t0, w2t0
                else:
                    w1t = []
                    for dc in range(DC):
                        t = w1p.tile([P, DF], BF16, tag="w1", name=f"w1_{k}_{dc}")
                        nc.sync.dma_start(t[:], w1b[k, dc * P:(dc + 1) * P, :])
                        w1t.append(t)
                    w2t8 = w2p.tile([P, FC, D], FP8, tag="w2", name=f"w2_{k}")
                    nc.sync.dma_start(w2t8[:], w2f[k])

                # L1 (bf16): psum = u/2  [f-major], then
                #   g = gelu(2*psum + b1)            (ACT, bf16)
                #   E8 = (g + (-mu)) - psum           (DVE stt, fp8) = E - mu
                ec = [
                    ep.tile([P, 2, T], FP8, tag="E", name=f"E_{k}_{c}")
                    for c in range(C8)
                ]
                for fc in range(FC):
                    for tcc in range(TC):
                        ph = psA.tile([P, 512], F32, tag="ph",
                                      name=f"ph_{k}_{fc}_{tcc}")
                        for dc in range(DC):
                            nc.tensor.matmul(
                                ph[:],
                                w1t[dc][:, fc * P:(fc + 1) * P],
                                xt[dc][:, tcc * 512:(tcc + 1) * 512],
                                start=(dc == 0),
                                stop=(dc == DC - 1),
                            )
                        g = gp.tile([P, 512], BF16, tag="g",
                                    name=f"g_{k}_{fc}_{tcc}")
                        nc.scalar.activation(
                            g[:], ph[:], gelu_fn,
                            bias=b1_sb[k][:, fc:fc + 1], scale=1.0 / S_W1,
                        )
                        nc.vector.scalar_tensor_tensor(
                            ec[fc // 2][:, (fc % 2):(fc % 2) + 1,
                                        tcc * 512:(tcc + 1) * 512],
                            g[:], nmu_sb[:, k:k + 1], ph[:],
                            op0=add, op1=mybir.AluOpType.subtract,
                        )

                # combine: po = x@M (bf16) + E8 @ w2 (fp8 DoubleRow)
                #          acc (+)= po * gw
                for tt in range(TT):
                    po = psB.tile([P, 512], F32, tag="po", name=f"po_{k}_{tt}")
                    for dc in range(DC):
                        nc.tensor.matmul(
                            po[:, 0:D],
                            xt[dc][:, tt * P:(tt + 1) * P],
                            m_sb[k][:, dc:dc + 1, :],
                            start=(dc == 0),
                            stop=False,
                        )
                    for c in range(C8):
                        nc.tensor.matmul(
                            po[:, 0:D],
                            ec[c][:, :, tt * P:(tt + 1) * P],
                            w2t8[:, 2 * c:2 * c + 2, :],
                            start=False,
                            stop=(c == C8 - 1),
                            perf_mode=DR,
                        )
                    if k == 0:
                        nc.vector.tensor_scalar_mul(
                            acc[tt][:], po[:, 0:D], gw_sb[tt][:, 0:1]
                        )
                    else:
                        nc.vector.scalar_tensor_tensor(
                            acc[tt][:], po[:, 0:D], gw_sb[tt][:, k:k + 1],
                            acc[tt][:], op0=mult, op1=add,
                        )
                    if k == K - 1:
                        nc.sync.dma_start(
                            out[tt * P:(tt + 1) * P, :], acc[tt][:]
                        )

    _split_multi_waits(nc)
    return nc


# ---------------------------------------------------------------------------
# Host wrapper
# ---------------------------------------------------------------------------
_NC_CACHE: dict = {}


def _get_nc(K: int, T: int, D: int, DF: int):
    key = (K, T, D, DF)
    if key not in _NC_CACHE:
        _NC_CACHE[key] = build_moe_kernel(K, T, D, DF)
    return _NC_CACHE[key]


def _softmax(x, axis=-1):
    m = np.max(x, axis=axis, keepdims=True)
    e = np.exp(x - m)
    return e / np.sum(e, axis=axis, keepdims=True)


def _gelu_tanh_mean(sigma, b1):
    """E_z[gelu_tanh(b1 + sigma*z)], z~N(0,1), vectorized over features."""
    zg = np.linspace(-8.0, 8.0, 401)
    wg = np.exp(-0.5 * zg * zg)
    wg /= wg.sum()
    v = b1[:, None] + sigma[:, None] * zg[None, :]       # [DF, NZ]
    g = 0.5 * v * (1.0 + np.tanh(np.sqrt(2.0 / np.pi) * (v + 0.044715 * v**3)))
    return (g * wg[None, :]).sum(axis=1)                 # [DF]


def _f8(a):
    return np.clip(np.asarray(a, np.float32), -240.0, 240.0).astype(
        ml_dtypes.float8_e4m3fn
    )


def run(inputs: dict, trace: bool = False, tmpdir: str | None = None):
    x = np.asarray(inputs["x"], dtype=np.float32)
    gate_w = np.asarray(inputs["gate_w"], dtype=np.float32)
    gate_b = np.asarray(inputs["gate_b"], dtype=np.float32)
    w1 = np.asarray(inputs["w1"], dtype=np.float32)
    b1 = np.asarray(inputs["b1"], dtype=np.float32)
    w2 = np.asarray(inputs["w2"], dtype=np.float32)
    b2 = np.asarray(inputs["b2"], dtype=np.float32)
    K = int(inputs["num_available"])

    B, S, D = x.shape
    DF = w1.shape[2]
    Ttot = B * S
    T = Ttot // N_CORES
    DC = D // P
    FC = DF // P

    # Coarse routing on host (tiny): gate applied to the global token sum.
    ksum = x.sum(axis=(0, 1))
    coarse = gate_w @ ksum + gate_b
    idx = np.argsort(-coarse, kind="stable")[:K]

    gws, gbs = gate_w[idx], gate_b[idx]
    w1s = np.ascontiguousarray(w1[idx])                  # [K,D,DF] f32
    b1s = np.ascontiguousarray(b1[idx], dtype=np.float32)
    w2s = np.ascontiguousarray(w2[idx])                  # [K,DF,D] f32
    b2s = np.ascontiguousarray(b2[idx], dtype=np.float32)

    # device tensors
    w1b = (S_W1 * w1s).astype(ml_dtypes.bfloat16)        # [K,D,DF]
    w2f = np.ascontiguousarray(
        _f8(S_W2 * w2s).reshape(K, FC, P, D).transpose(0, 2, 1, 3)
    )                                                    # [K,P,FC,D] fp8
    mB = (S_M * np.matmul(w1s, w2s)).astype(ml_dtypes.bfloat16)  # [K,D,D]

    # E-residual means (exact Gaussian statistics: u_f ~ N(0, ||w1[:,f]||)).
    mus = np.empty(K, np.float32)
    for k in range(K):
        sigma = np.linalg.norm(w1s[k], axis=0)           # [DF]
        mus[k] = _gelu_tanh_mean(sigma, b1s[k]).mean() - 0.0
    nmu = np.ascontiguousarray(
        np.broadcast_to(-mus[None, :], (P, K)), dtype=np.float32
    )

    gwsT = np.ascontiguousarray(
        gws.T.reshape(DC, P, K).transpose(1, 0, 2).reshape(P, DC * K)
    ).astype(ml_dtypes.bfloat16)
    gbb = np.ascontiguousarray(np.broadcast_to(gbs[None, :], (P, K)),
                               dtype=np.float32)
    b1p = np.ascontiguousarray(
        b1s.reshape(K, FC, P).transpose(0, 2, 1), dtype=np.float32
    )

    xf = x.reshape(Ttot, D)
    xT_bf = np.ascontiguousarray(xf.T).astype(ml_dtypes.bfloat16)

    nc = _get_nc(K, T, D, DF)
    in_maps = []
    for c in range(N_CORES):
        in_maps.append({
            "xT": np.ascontiguousarray(xT_bf[:, c * T:(c + 1) * T]),
            "w1b": w1b,
            "w2f": w2f,
            "mB": mB,
            "gwsT": gwsT,
            "gbb": gbb,
            "b1p": b1p,
            "nmu": nmu,
        })

    res = run_bass_kernel_spmd(
        nc, in_maps, list(range(N_CORES)), trace=trace, tmpdir=tmpdir
    )
    outp = np.concatenate(
        [res.results[c]["out"] for c in range(N_CORES)], axis=0
    ).reshape(B, S, D).astype(np.float32)

    # Exact rank-1 correction: sum_k gw_k * (mu_k*colsum(W2_k) + b2_k).
    # gw recomputed on host in fp32; mismatch vs device bf16 gw is O(1e-5).
    C = mus[:, None] * w2s.sum(axis=1) + b2s             # [K, D]
    logits = xf @ gws.T + gbs[None, :]
    gwh = _softmax(logits, axis=1).astype(np.float32)
    outp = outp + (gwh @ C).reshape(B, S, D)

    return outp, res


def kernel(**inputs) -> np.ndarray:
    outp, _ = run(inputs, trace=False)
    return outp


# revision 15
# speedup vs baseline: 1.2192x; 1.0148x over previous
"""Mixture-of-Experts Trainium2 kernel (8-core SPMD, token-sharded).

Reference: coarse top-K expert selection from the gate applied to the global
token sum, then dense K-expert FFN over all tokens with per-token softmax
gating over the K selected experts.

Hybrid-precision device strategy ("E-split"). The pointwise identity
    gelu(v) = v/2 + E(v),   E even, small (std ~0.1)
splits each expert's FFN into an exact linear path and a residual path:
    gelu(x@W1 + b1) @ W2 = x @ (W1@W2)/2 + (b1/2 + E(v)) @ W2
The linear path is a cheap [D,D] bf16 matmul (precomputed W1@W2 on host).
Only the residual E - mu (mean-removed, quantized fp8e4) goes through the
big [DF,D] contraction, which then runs as fp8 DoubleRow matmuls at ~1.8x
the bf16 rate.  L1 (x@W1, needed to evaluate E) stays bf16 for accuracy.
The exact rank-1 term sum_k gw_k * (mu_k*colsum(W2_k) + b2_k) is added on
host (gw recomputed exactly there; mismatch vs device gw is ~1e-5).

Scales: w1 x32 (psum=32u, gelu scale 1/32), E x4, w2 x32, lin M = 64*W1W2
=> combine psum = 128 * eo; softmax gating weights pre-divided by 128.

Per core (T=2048 tokens): gating bf16 (token-major), then per expert:
L1 64 psum tiles (4 bf16 MMs each) -> gelu (ACT) -> E fp8 (2 DVE ops),
combine 16 psum tiles (4 bf16 lin MMs + 8 fp8-DR MMs) -> acc (+)= po*gw.
Output stores stream per-tile during the last expert's combine.
"""

import numpy as np
import ml_dtypes
from contextlib import ExitStack

import bass_rust as _bass_rust
import concourse.bass as bass
import concourse.mybir as mybir
import concourse.tile as tile
from concourse.bass_utils import run_bass_kernel_spmd

BF16 = mybir.dt.bfloat16
FP8 = mybir.dt.float8e4
F32 = mybir.dt.float32
N_CORES = 8
P = 128

S_W1 = 0.5     # w1 pre-scale (L1 psum = u/2, so E = g - psum in ONE DVE op)
S_W2 = 128.0   # w2 pre-scale
S_PS = S_W2                # combine psum scale (=128)
S_M = S_PS / 2.0           # lin path: M = 64 * (W1 @ W2)


# ---------------------------------------------------------------------------
# Workaround for walrus "Too many sync wait commands": this walrus build
# accepts at most one semaphore wait in a single instruction's sync_info,
# but Tile's scheduler (and its kernel-tail drain) can attach several.
# Post-pass: move excess waits onto standalone EventSemaphore instructions
# inserted immediately before the offender on the same engine.
# ---------------------------------------------------------------------------
_split_ctr = [0]


def _split_multi_waits(nc):
    for f in nc.m.functions:
        for blk in f.blocks:
            insts = blk.instructions
            i = 0
            while i < len(insts):
                inst = insts[i]
                si = getattr(inst, "sync_info", None)
                waits = list(si.on_wait) if si is not None and si.on_wait else []
                if len(waits) > 1:
                    si.on_wait = waits[-1:]
                    for w in waits[:-1]:
                        _split_ctr[0] += 1
                        ev = mybir.InstEventSemaphore(
                            name=f"I-wsplit-{_split_ctr[0]}", ins=[], outs=[]
                        )
                        ev.engine = inst.engine
                        ev.sync_info = _bass_rust.SyncInfo(
                            on_wait=[w], on_update=[]
                        )
                        insts.insert(i, ev)
                        i += 1
                i += 1


# ---------------------------------------------------------------------------
# Device kernel
# ---------------------------------------------------------------------------
def build_moe_kernel(K: int, T: int, D: int, DF: int):
    assert T % 512 == 0 and D % P == 0 and DF % 256 == 0
    TT = T // P       # 128-token tiles
    TC = T // 512     # 512-token chunks
    DC = D // P       # D chunks of 128
    FC = DF // P      # F chunks of 128
    C8 = DF // 256    # DoubleRow chunks (256-deep each)

    nc = bass.Bass("TRN2", target_bir_lowering=False)

    xT = nc.declare_dram_parameter("xT", [D, T], BF16, isOutput=False)
    w1b = nc.declare_dram_parameter("w1b", [K, D, DF], BF16, isOutput=False)
    w2f = nc.declare_dram_parameter("w2f", [K, P, FC, D], FP8, isOutput=False)
    mB = nc.declare_dram_parameter("mB", [K, D, D], BF16, isOutput=False)
    gwsT = nc.declare_dram_parameter("gwsT", [P, DC * K], BF16, isOutput=False)
    gbb = nc.declare_dram_parameter("gbb", [P, K], F32, isOutput=False)
    b1p = nc.declare_dram_parameter("b1p", [K, P, FC], F32, isOutput=False)
    nmu = nc.declare_dram_parameter("nmu", [P, K], F32, isOutput=False)
    out = nc.declare_dram_parameter("out", [T, D], F32, isOutput=True)

    mult = mybir.AluOpType.mult
    add = mybir.AluOpType.add
    gelu_fn = mybir.ActivationFunctionType.Gelu_apprx_tanh
    exp_fn = mybir.ActivationFunctionType.Exp
    DR = mybir.MatmulPerfMode.DoubleRow

    with tile.TileContext(nc) as tc:
        with ExitStack() as ctx:
            persist = ctx.enter_context(tc.tile_pool(name="persist", bufs=1))
            w1p = ctx.enter_context(tc.tile_pool(name="w1p", bufs=2 * DC))
            w2p = ctx.enter_context(tc.tile_pool(name="w2p", bufs=2))
            ep = ctx.enter_context(tc.tile_pool(name="ep", bufs=C8))
            gp = ctx.enter_context(tc.tile_pool(name="gp", bufs=3))
            sm = ctx.enter_context(tc.tile_pool(name="sm", bufs=4))
            psA = ctx.enter_context(tc.tile_pool(name="psA", bufs=4, space="PSUM"))
            psB = ctx.enter_context(tc.tile_pool(name="psB", bufs=4, space="PSUM"))

            # ---- persistent loads. Emission order = DMA queue order:
            # tiny tensors first (gating needs gws immediately), then x
            # chunks interleaved with expert-0 w1 chunks so gating can start
            # on x chunk 0 while w1 still streams. ----
            gws_sb = persist.tile([P, DC * K], BF16, tag="gws", name="gws_sb")
            nc.sync.dma_start(gws_sb[:], gwsT[:])
            gbb_sb = persist.tile([P, K], F32, tag="gbb", name="gbb_sb")
            nc.sync.dma_start(gbb_sb[:], gbb[:])
            nmu_sb = persist.tile([P, K], F32, tag="nmu", name="nmu_sb")
            nc.sync.dma_start(nmu_sb[:], nmu[:])
            b1_sb = []
            for k in range(K):
                t = persist.tile([P, FC], F32, tag=f"b1_{k}", name=f"b1_{k}")
                nc.sync.dma_start(t[:], b1p[k])
                b1_sb.append(t)

            xt = [
                persist.tile([P, T], BF16, tag=f"xt{dc}", name=f"xt{dc}")
                for dc in range(DC)
            ]
            w1t0 = [
                w1p.tile([P, DF], BF16, tag="w1", name=f"w1_0_{dc}")
                for dc in range(DC)
            ]
            for dc in range(DC):
                nc.sync.dma_start(xt[dc][:], xT[dc * P:(dc + 1) * P, :])
            for dc in range(DC):
                nc.sync.dma_start(w1t0[dc][:], w1b[0, dc * P:(dc + 1) * P, :])
            w2t0 = w2p.tile([P, FC, D], FP8, tag="w2", name="w2_0")
            nc.sync.dma_start(w2t0[:], w2f[0])

            # lin-path moving matrices, all experts resident
            m_sb = []
            for k in range(K):
                t = persist.tile([P, DC, D], BF16, tag=f"m{k}", name=f"m{k}")
                for dc in range(DC):
                    nc.sync.dma_start(
                        t[:, dc:dc + 1, :], mB[k, dc * P:(dc + 1) * P, :]
                    )
                m_sb.append(t)

            acc = [
                persist.tile([P, D], F32, tag=f"acc{t}", name=f"acc{t}")
                for t in range(TT)
            ]
            gw_sb = [
                persist.tile([P, K], F32, tag=f"gw{t}", name=f"gw{t}")
                for t in range(TT)
            ]

            # ---- gating softmax (token-major); gw_sb holds softmax/S_PS ----
            # dc-outer rounds of 4 token tiles: the first matmuls only need
            # x chunk 0, so gating starts while the rest of x still streams.
            RT = 4
            for rnd in range(0, TT, RT):
                tts = range(rnd, min(rnd + RT, TT))
                pls = {}
                for tt in tts:
                    pls[tt] = psB.tile([P, 512], F32, tag="po", name=f"pl{tt}")
                for dc in range(DC):
                    for tt in tts:
                        nc.tensor.matmul(
                            pls[tt][:, 0:K],
                            xt[dc][:, tt * P:(tt + 1) * P],
                            gws_sb[:, dc * K:(dc + 1) * K],
                            start=(dc == 0),
                            stop=(dc == DC - 1),
                        )
                for tt in tts:
                    pl = pls[tt]
                    l_sb = sm.tile([P, K], F32, tag="l", name=f"l{tt}")
                    nc.vector.tensor_add(l_sb[:], pl[:, 0:K], gbb_sb[:])
                    negmax = sm.tile([P, 1], F32, tag="negmax",
                                     name=f"negmax{tt}")
                    nc.vector.reduce_max(
                        negmax[:], l_sb[:], axis=mybir.AxisListType.X,
                        negate=True,
                    )
                    z = sm.tile([P, 1], F32, tag="z", name=f"z{tt}")
                    nc.scalar.activation(
                        gw_sb[tt][:], l_sb[:], exp_fn,
                        bias=negmax[:, 0:1], accum_out=z[:, 0:1],
                    )
                    rz = sm.tile([P, 1], F32, tag="rz", name=f"rz{tt}")
                    nc.vector.reciprocal(rz[:], z[:, 0:1])
                    nc.vector.tensor_scalar(
                        gw_sb[tt][:], gw_sb[tt][:], rz[:, 0:1], 1.0 / S_PS,
                        op0=mult, op1=mult,
                    )

            # ---- experts ----
            for k in range(K):
                if k == 0:
                    w1t, w2t8 = w1t0, w2t0
                else:
                    w1t = []
                    for dc in range(DC):
                        t = w1p.tile([P, DF], BF16, tag="w1", name=f"w1_{k}_{dc}")
                        nc.sync.dma_start(t[:], w1b[k, dc * P:(dc + 1) * P, :])
                        w1t.append(t)
                    w2t8 = w2p.tile([P, FC, D], FP8, tag="w2", name=f"w2_{k}")
                    nc.sync.dma_start(w2t8[:], w2f[k])

                # L1 (bf16): psum = u/2  [f-major], then
                #   g = gelu(2*psum + b1)            (ACT, bf16)
                #   E8 = (g + (-mu)) - psum           (DVE stt, fp8) = E - mu
                ec = [
                    ep.tile([P, 2, T], FP8, tag="E", name=f"E_{k}_{c}")
                    for c in range(C8)
                ]
                for fc in range(FC):
                    for tcc in range(TC):
                        ph = psA.tile([P, 512], F32, tag="ph",
                                      name=f"ph_{k}_{fc}_{tcc}")
                        for dc in range(DC):
                            nc.tensor.matmul(
                                ph[:],
                                w1t[dc][:, fc * P:(fc + 1) * P],
                                xt[dc][:, tcc * 512:(tcc + 1) * 512],
                                start=(dc == 0),
                                stop=(dc == DC - 1),
                            )
                        g = gp.tile([P, 512], BF16, tag="g",
                                    name=f"g_{k}_{fc}_{tcc}")
                        nc.scalar.activation(
                            g[:], ph[:], gelu_fn,
                            bias=b1_sb[k][:, fc:fc + 1], scale=1.0 / S_W1,
                        )
                        nc.vector.scalar_tensor_tensor(
                            ec[fc // 2][:, (fc % 2):(fc % 2) + 1,
                                        tcc * 512:(tcc + 1) * 512],
                            g[:], nmu_sb[:, k:k + 1], ph[:],
                            op0=add, op1=mybir.AluOpType.subtract,
                        )

                # combine: po = x@M (bf16) + E8 @ w2 (fp8 DoubleRow)
                #          acc (+)= po * gw
                for tt in range(TT):
                    po = psB.tile([P, 512], F32, tag="po", name=f"po_{k}_{tt}")
                    for dc in range(DC):
                        nc.tensor.matmul(
                            po[:, 0:D],
                            xt[dc][:, tt * P:(tt + 1) * P],
                            m_sb[k][:, dc:dc + 1, :],
                            start=(dc == 0),
                            stop=False,
                        )
                    for c in range(C8):
                        nc.tensor.matmul(
                            po[:, 0:D],
                            ec[c][:, :, tt * P:(tt + 1) * P],
                            w2t8[:, 2 * c:2 * c + 2, :],
                            start=False,
                            stop=(c == C8 - 1),
                            perf_mode=DR,
                        )
                    if k == 0:
                        nc.vector.tensor_scalar_mul(
                            acc[tt][:], po[:, 0:D], gw_sb[tt][:, 0:1]
                        )
                    else:
                        nc.vector.scalar_tensor_tensor(
                            acc[tt][:], po[:, 0:D], gw_sb[tt][:, k:k + 1],
                            acc[tt][:], op0=mult, op1=add,
                        )
                    if k == K - 1:
                        nc.sync.dma_start(
                            out[tt * P:(tt + 1) * P, :], acc[tt][:]
                        )

    _split_multi_waits(nc)
    return nc


# ---------------------------------------------------------------------------
# Host wrapper
# ---------------------------------------------------------------------------
_NC_CACHE: dict = {}


def _get_nc(K: int, T: int, D: int, DF: int):
    key = (K, T, D, DF)
    if key not in _NC_CACHE:
        _NC_CACHE[key] = build_moe_kernel(K, T, D, DF)
    return _NC_CACHE[key]


def _softmax(x, axis=-1):
    m = np.max(x, axis=axis, keepdims=True)
    e = np.exp(x - m)
    return e / np.sum(e, axis=axis, keepdims=True)


def _gelu_tanh_mean(sigma, b1):
    """E_z[gelu_tanh(b1 + sigma*z)], z~N(0,1), vectorized over features."""
    zg = np.linspace(-8.0, 8.0, 401)
    wg = np.exp(-0.5 * zg * zg)
    wg /= wg.sum()
    v = b1[:, None] + sigma[:, None] * zg[None, :]       # [DF, NZ]
    g = 0.5 * v * (1.0 + np.tanh(np.sqrt(2.0 / np.pi) * (v + 0.044715 * v**3)))
    return (g * wg[None, :]).sum(axis=1)                 # [DF]


def _f8(a):
    return np.clip(np.asarray(a, np.float32), -240.0, 240.0).astype(
        ml_dtypes.float8_e4m3fn
    )


def run(inputs: dict, trace: bool = False, tmpdir: str | None = None):
    x = np.asarray(inputs["x"], dtype=np.float32)
    gate_w = np.asarray(inputs["gate_w"], dtype=np.float32)
    gate_b = np.asarray(inputs["gate_b"], dtype=np.float32)
    w1 = np.asarray(inputs["w1"], dtype=np.float32)
    b1 = np.asarray(inputs["b1"], dtype=np.float32)
    w2 = np.asarray(inputs["w2"], dtype=np.float32)
    b2 = np.asarray(inputs["b2"], dtype=np.float32)
    K = int(inputs["num_available"])

    B, S, D = x.shape
    DF = w1.shape[2]
    Ttot = B * S
    T = Ttot // N_CORES
    DC = D // P
    FC = DF // P

    # Coarse routing on host (tiny): gate applied to the global token sum.
    ksum = x.sum(axis=(0, 1))
    coarse = gate_w @ ksum + gate_b
    idx = np.argsort(-coarse, kind="stable")[:K]

    gws, gbs = gate_w[idx], gate_b[idx]
    w1s = np.ascontiguousarray(w1[idx])                  # [K,D,DF] f32
    b1s = np.ascontiguousarray(b1[idx], dtype=np.float32)
    w2s = np.ascontiguousarray(w2[idx])                  # [K,DF,D] f32
    b2s = np.ascontiguousarray(b2[idx], dtype=np.float32)

    # device tensors
    w1b = (S_W1 * w1s).astype(ml_dtypes.bfloat16)        # [K,D,DF]
    w2f = np.ascontiguousarray(
        _f8(S_W2 * w2s).reshape(K, FC, P, D).transpose(0, 2, 1, 3)
    )                                                    # [K,P,FC,D] fp8
    mB = (S_M * np.matmul(w1s, w2s)).astype(ml_dtypes.bfloat16)  # [K,D,D]

    # E-residual means (exact Gaussian statistics: u_f ~ N(0, ||w1[:,f]||)).
    mus = np.empty(K, np.float32)
    for k in range(K):
        sigma = np.linalg.norm(w1s[k], axis=0)           # [DF]
        mus[k] = _gelu_tanh_mean(sigma, b1s[k]).mean() - 0.0
    nmu = np.ascontiguousarray(
        np.broadcast_to(-mus[None, :], (P, K)), dtype=np.float32
    )

    gwsT = np.ascontiguousarray(
        gws.T.reshape(DC, P, K).transpose(1, 0, 2).reshape(P, DC * K)
    ).astype(ml_dtypes.bfloat16)
    gbb = np.ascontiguousarray(np.broadcast_to(gbs[None, :], (P, K)),
                               dtype=np.float32)
    b1p = np.ascontiguousarray(
        b1s.reshape(K, FC, P).transpose(0, 2, 1), dtype=np.float32
    )

    xf = x.reshape(Ttot, D)
    xT_bf = np.ascontiguousarray(xf.T).astype(ml_dtypes.bfloat16)

    nc = _get_nc(K, T, D, DF)
    in_maps = []
    for c in range(N_CORES):
        in_maps.append({
            "xT": np.ascontiguousarray(xT_bf[:, c * T:(c + 1) * T]),
            "w1b": w1b,
            "w2f": w2f,
            "mB": mB,
            "gwsT": gwsT,
            "gbb": gbb,
            "b1p": b1p,
            "nmu": nmu,
        })

    res = run_bass_kernel_spmd(
        nc, in_maps, list(range(N_CORES)), trace=trace, tmpdir=tmpdir
    )
    outp = np.concatenate(
        [res.results[c]["out"] for c in range(N_CORES)], axis=0
    ).reshape(B, S, D).astype(np.float32)

    # Exact rank-1 correction: sum_k gw_k * (mu_k*colsum(W2_k) + b2_k).
    # gw recomputed on host in fp32; mismatch vs device bf16 gw is O(1e-5).
    C = mus[:, None] * w2s.sum(axis=1) + b2s             # [K, D]
    logits = xf @ gws.T + gbs[None, :]
    gwh = _softmax(logits, axis=1).astype(np.float32)
    outp = outp + (gwh @ C).reshape(B, S, D)

    return outp, res


def kernel(**inputs) -> np.ndarray:
    outp, _ = run(inputs, trace=False)
    return outp


# revision 16
# speedup vs baseline: 1.2267x; 1.0062x over previous
"""Mixture-of-Experts Trainium2 kernel (8-core SPMD, token-sharded).

Reference: coarse top-K expert selection from the gate applied to the global
token sum, then dense K-expert FFN over all tokens with per-token softmax
gating over the K selected experts.

Hybrid-precision device strategy ("E-split"). The pointwise identity
    gelu(v) = v/2 + E(v),   E even, small (std ~0.1)
splits each expert's FFN into an exact linear path and a residual path:
    gelu(x@W1 + b1) @ W2 = x @ (W1@W2)/2 + (b1/2 + E(v)) @ W2
The linear path is a cheap [D,D] bf16 matmul (precomputed W1@W2 on host).
Only the residual E - mu (mean-removed, quantized fp8e4) goes through the
big [DF,D] contraction, which then runs as fp8 DoubleRow matmuls at ~1.8x
the bf16 rate.  L1 (x@W1, needed to evaluate E) stays bf16 for accuracy.
The exact rank-1 term sum_k gw_k * (mu_k*colsum(W2_k) + b2_k) is added on
host (gw recomputed exactly there; mismatch vs device gw is ~1e-5).

Scales: w1 x32 (psum=32u, gelu scale 1/32), E x4, w2 x32, lin M = 64*W1W2
=> combine psum = 128 * eo; softmax gating weights pre-divided by 128.

Per core (T=2048 tokens): gating bf16 (token-major), then per expert:
L1 64 psum tiles (4 bf16 MMs each) -> gelu (ACT) -> E fp8 (2 DVE ops),
combine 16 psum tiles (4 bf16 lin MMs + 8 fp8-DR MMs) -> acc (+)= po*gw.
Output stores stream per-tile during the last expert's combine.
"""

import numpy as np
import ml_dtypes
from contextlib import ExitStack

import bass_rust as _bass_rust
import concourse.bass as bass
import concourse.mybir as mybir
import concourse.tile as tile
from concourse.bass_utils import run_bass_kernel_spmd

BF16 = mybir.dt.bfloat16
FP8 = mybir.dt.float8e4
F32 = mybir.dt.float32
N_CORES = 8
P = 128

S_W1 = 0.5     # w1 pre-scale (L1 psum = u/2, so E = g - psum in ONE DVE op)
S_W2 = 128.0   # w2 pre-scale
S_PS = S_W2                # combine psum scale (=128)
S_M = S_PS / 2.0           # lin path: M = 64 * (W1 @ W2)


# ---------------------------------------------------------------------------
# Workaround for walrus "Too many sync wait commands": this walrus build
# accepts at most one semaphore wait in a single instruction's sync_info,
# but Tile's scheduler (and its kernel-tail drain) can attach several.
# Post-pass: move excess waits onto standalone EventSemaphore instructions
# inserted immediately before the offender on the same engine.
# ---------------------------------------------------------------------------
_split_ctr = [0]


def _split_multi_waits(nc):
    for f in nc.m.functions:
        for blk in f.blocks:
            insts = blk.instructions
            i = 0
            while i < len(insts):
                inst = insts[i]
                si = getattr(inst, "sync_info", None)
                waits = list(si.on_wait) if si is not None and si.on_wait else []
                if len(waits) > 1:
                    si.on_wait = waits[-1:]
                    for w in waits[:-1]:
                        _split_ctr[0] += 1
                        ev = mybir.InstEventSemaphore(
                            name=f"I-wsplit-{_split_ctr[0]}", ins=[], outs=[]
                        )
                        ev.engine = inst.engine
                        ev.sync_info = _bass_rust.SyncInfo(
                            on_wait=[w], on_update=[]
                        )
                        insts.insert(i, ev)
                        i += 1
                i += 1


# ---------------------------------------------------------------------------
# Device kernel
# ---------------------------------------------------------------------------
def build_moe_kernel(K: int, T: int, D: int, DF: int):
    assert T % 512 == 0 and D % P == 0 and DF % 256 == 0
    TT = T // P       # 128-token tiles
    TC = T // 512     # 512-token chunks
    DC = D // P       # D chunks of 128
    FC = DF // P      # F chunks of 128
    C8 = DF // 256    # DoubleRow chunks (256-deep each)

    nc = bass.Bass("TRN2", target_bir_lowering=False)

    xT = nc.declare_dram_parameter("xT", [D, T], BF16, isOutput=False)
    w1b = nc.declare_dram_parameter("w1b", [K, D, DF], BF16, isOutput=False)
    w2f = nc.declare_dram_parameter("w2f", [K, P, FC, D], FP8, isOutput=False)
    mB = nc.declare_dram_parameter("mB", [K, D, D], BF16, isOutput=False)
    gwsT = nc.declare_dram_parameter("gwsT", [P, DC * K], BF16, isOutput=False)
    gbb = nc.declare_dram_parameter("gbb", [P, K], F32, isOutput=False)
    b1p = nc.declare_dram_parameter("b1p", [K, P, FC], F32, isOutput=False)
    nmu = nc.declare_dram_parameter("nmu", [P, K], F32, isOutput=False)
    out = nc.declare_dram_parameter("out", [T, D], F32, isOutput=True)

    mult = mybir.AluOpType.mult
    add = mybir.AluOpType.add
    gelu_fn = mybir.ActivationFunctionType.Gelu_apprx_tanh
    exp_fn = mybir.ActivationFunctionType.Exp
    DR = mybir.MatmulPerfMode.DoubleRow

    with tile.TileContext(nc) as tc:
        with ExitStack() as ctx:
            persist = ctx.enter_context(tc.tile_pool(name="persist", bufs=1))
            w1p = ctx.enter_context(tc.tile_pool(name="w1p", bufs=2 * DC))
            w2p = ctx.enter_context(tc.tile_pool(name="w2p", bufs=2))
            ep = ctx.enter_context(tc.tile_pool(name="ep", bufs=C8))
            gp = ctx.enter_context(tc.tile_pool(name="gp", bufs=6))
            sm = ctx.enter_context(tc.tile_pool(name="sm", bufs=4))
            psA = ctx.enter_context(tc.tile_pool(name="psA", bufs=4, space="PSUM"))
            psB = ctx.enter_context(tc.tile_pool(name="psB", bufs=4, space="PSUM"))

            # ---- persistent loads. Emission order = DMA queue order:
            # tiny tensors first (gating needs gws immediately), then x
            # chunks interleaved with expert-0 w1 chunks so gating can start
            # on x chunk 0 while w1 still streams. ----
            gws_sb = persist.tile([P, DC * K], BF16, tag="gws", name="gws_sb")
            nc.sync.dma_start(gws_sb[:], gwsT[:])
            gbb_sb = persist.tile([P, K], F32, tag="gbb", name="gbb_sb")
            nc.sync.dma_start(gbb_sb[:], gbb[:])
            nmu_sb = persist.tile([P, K], F32, tag="nmu", name="nmu_sb")
            nc.sync.dma_start(nmu_sb[:], nmu[:])
            b1_sb = []
            for k in range(K):
                t = persist.tile([P, FC], F32, tag=f"b1_{k}", name=f"b1_{k}")
                nc.sync.dma_start(t[:], b1p[k])
                b1_sb.append(t)

            xt = [
                persist.tile([P, T], BF16, tag=f"xt{dc}", name=f"xt{dc}")
                for dc in range(DC)
            ]
            w1t0 = [
                w1p.tile([P, DF], BF16, tag="w1", name=f"w1_0_{dc}")
                for dc in range(DC)
            ]
            for dc in range(DC):
                nc.sync.dma_start(xt[dc][:], xT[dc * P:(dc + 1) * P, :])
            for dc in range(DC):
                nc.sync.dma_start(w1t0[dc][:], w1b[0, dc * P:(dc + 1) * P, :])
            w2t0 = w2p.tile([P, FC, D], FP8, tag="w2", name="w2_0")
            nc.sync.dma_start(w2t0[:], w2f[0])

            # lin-path moving matrices, all experts resident
            m_sb = []
            for k in range(K):
                t = persist.tile([P, DC, D], BF16, tag=f"m{k}", name=f"m{k}")
                for dc in range(DC):
                    nc.sync.dma_start(
                        t[:, dc:dc + 1, :], mB[k, dc * P:(dc + 1) * P, :]
                    )
                m_sb.append(t)

            acc = [
                persist.tile([P, D], F32, tag=f"acc{t}", name=f"acc{t}")
                for t in range(TT)
            ]
            gw_sb = [
                persist.tile([P, K], F32, tag=f"gw{t}", name=f"gw{t}")
                for t in range(TT)
            ]

            # ---- gating softmax (token-major); gw_sb holds softmax/S_PS ----
            # dc-outer rounds of 4 token tiles: the first matmuls only need
            # x chunk 0, so gating starts while the rest of x still streams.
            RT = 4
            for rnd in range(0, TT, RT):
                tts = range(rnd, min(rnd + RT, TT))
                pls = {}
                for tt in tts:
                    pls[tt] = psB.tile([P, 512], F32, tag="po", name=f"pl{tt}")
                for dc in range(DC):
                    for tt in tts:
                        nc.tensor.matmul(
                            pls[tt][:, 0:K],
                            xt[dc][:, tt * P:(tt + 1) * P],
                            gws_sb[:, dc * K:(dc + 1) * K],
                            start=(dc == 0),
                            stop=(dc == DC - 1),
                        )
                for tt in tts:
                    pl = pls[tt]
                    l_sb = sm.tile([P, K], F32, tag="l", name=f"l{tt}")
                    nc.vector.tensor_add(l_sb[:], pl[:, 0:K], gbb_sb[:])
                    negmax = sm.tile([P, 1], F32, tag="negmax",
                                     name=f"negmax{tt}")
                    nc.vector.reduce_max(
                        negmax[:], l_sb[:], axis=mybir.AxisListType.X,
                        negate=True,
                    )
                    z = sm.tile([P, 1], F32, tag="z", name=f"z{tt}")
                    nc.scalar.activation(
                        gw_sb[tt][:], l_sb[:], exp_fn,
                        bias=negmax[:, 0:1], accum_out=z[:, 0:1],
                    )
                    rz = sm.tile([P, 1], F32, tag="rz", name=f"rz{tt}")
                    nc.vector.reciprocal(rz[:], z[:, 0:1])
                    nc.vector.tensor_scalar(
                        gw_sb[tt][:], gw_sb[tt][:], rz[:, 0:1], 1.0 / S_PS,
                        op0=mult, op1=mult,
                    )

            # ---- experts ----
            for k in range(K):
                if k == 0:
                    w1t, w2t8 = w1t0, w2t0
                else:
                    w1t = []
                    for dc in range(DC):
                        t = w1p.tile([P, DF], BF16, tag="w1", name=f"w1_{k}_{dc}")
                        nc.sync.dma_start(t[:], w1b[k, dc * P:(dc + 1) * P, :])
                        w1t.append(t)
                    w2t8 = w2p.tile([P, FC, D], FP8, tag="w2", name=f"w2_{k}")
                    nc.sync.dma_start(w2t8[:], w2f[k])

                # L1 (bf16): psum = u/2  [f-major], then
                #   g = gelu(2*psum + b1)            (ACT, bf16)
                #   E8 = (g + (-mu)) - psum           (DVE stt, fp8) = E - mu
                ec = [
                    ep.tile([P, 2, T], FP8, tag="E", name=f"E_{k}_{c}")
                    for c in range(C8)
                ]
                for fc in range(FC):
                    for tcc in range(TC):
                        ph = psA.tile([P, 512], F32, tag="ph",
                                      name=f"ph_{k}_{fc}_{tcc}")
                        for dc in range(DC):
                            nc.tensor.matmul(
                                ph[:],
                                w1t[dc][:, fc * P:(fc + 1) * P],
                                xt[dc][:, tcc * 512:(tcc + 1) * 512],
                                start=(dc == 0),
                                stop=(dc == DC - 1),
                            )
                        g = gp.tile([P, 512], BF16, tag="g",
                                    name=f"g_{k}_{fc}_{tcc}")
                        nc.scalar.activation(
                            g[:], ph[:], gelu_fn,
                            bias=b1_sb[k][:, fc:fc + 1], scale=1.0 / S_W1,
                        )
                        nc.vector.scalar_tensor_tensor(
                            ec[fc // 2][:, (fc % 2):(fc % 2) + 1,
                                        tcc * 512:(tcc + 1) * 512],
                            g[:], nmu_sb[:, k:k + 1], ph[:],
                            op0=add, op1=mybir.AluOpType.subtract,
                        )

                # combine: po = x@M (bf16) + E8 @ w2 (fp8 DoubleRow)
                #          acc (+)= po * gw
                for tt in range(TT):
                    po = psB.tile([P, 512], F32, tag="po", name=f"po_{k}_{tt}")
                    for dc in range(DC):
                        nc.tensor.matmul(
                            po[:, 0:D],
                            xt[dc][:, tt * P:(tt + 1) * P],
                            m_sb[k][:, dc:dc + 1, :],
                            start=(dc == 0),
                            stop=False,
                        )
                    for c in range(C8):
                        nc.tensor.matmul(
                            po[:, 0:D],
                            ec[c][:, :, tt * P:(tt + 1) * P],
                            w2t8[:, 2 * c:2 * c + 2, :],
                            start=False,
                            stop=(c == C8 - 1),
                            perf_mode=DR,
                        )
                    if k == 0:
                        nc.vector.tensor_scalar_mul(
                            acc[tt][:], po[:, 0:D], gw_sb[tt][:, 0:1]
                        )
                    else:
                        nc.vector.scalar_tensor_tensor(
                            acc[tt][:], po[:, 0:D], gw_sb[tt][:, k:k + 1],
                            acc[tt][:], op0=mult, op1=add,
                        )
                    if k == K - 1:
                        nc.sync.dma_start(
                            out[tt * P:(tt + 1) * P, :], acc[tt][:]
                        )

    _split_multi_waits(nc)
    return nc


# ---------------------------------------------------------------------------
# Host wrapper
# ---------------------------------------------------------------------------
_NC_CACHE: dict = {}


def _get_nc(K: int, T: int, D: int, DF: int):
    key = (K, T, D, DF)
    if key not in _NC_CACHE:
        _NC_CACHE[key] = build_moe_kernel(K, T, D, DF)
    return _NC_CACHE[key]


def _softmax(x, axis=-1):
    m = np.max(x, axis=axis, keepdims=True)
    e = np.exp(x - m)
    return e / np.sum(e, axis=axis, keepdims=True)


def _gelu_tanh_mean(sigma, b1):
    """E_z[gelu_tanh(b1 + sigma*z)], z~N(0,1), vectorized over features."""
    zg = np.linspace(-8.0, 8.0, 401)
    wg = np.exp(-0.5 * zg * zg)
    wg /= wg.sum()
    v = b1[:, None] + sigma[:, None] * zg[None, :]       # [DF, NZ]
    g = 0.5 * v * (1.0 + np.tanh(np.sqrt(2.0 / np.pi) * (v + 0.044715 * v**3)))
    return (g * wg[None, :]).sum(axis=1)                 # [DF]


def _f8(a):
    return np.clip(np.asarray(a, np.float32), -240.0, 240.0).astype(
        ml_dtypes.float8_e4m3fn
    )


def run(inputs: dict, trace: bool = False, tmpdir: str | None = None):
    x = np.asarray(inputs["x"], dtype=np.float32)
    gate_w = np.asarray(inputs["gate_w"], dtype=np.float32)
    gate_b = np.asarray(inputs["gate_b"], dtype=np.float32)
    w1 = np.asarray(inputs["w1"], dtype=np.float32)
    b1 = np.asarray(inputs["b1"], dtype=np.float32)
    w2 = np.asarray(inputs["w2"], dtype=np.float32)
    b2 = np.asarray(inputs["b2"], dtype=np.float32)
    K = int(inputs["num_available"])

    B, S, D = x.shape
    DF = w1.shape[2]
    Ttot = B * S
    T = Ttot // N_CORES
    DC = D // P
    FC = DF // P

    # Coarse routing on host (tiny): gate applied to the global token sum.
    ksum = x.sum(axis=(0, 1))
    coarse = gate_w @ ksum + gate_b
    idx = np.argsort(-coarse, kind="stable")[:K]

    gws, gbs = gate_w[idx], gate_b[idx]
    w1s = np.ascontiguousarray(w1[idx])                  # [K,D,DF] f32
    b1s = np.ascontiguousarray(b1[idx], dtype=np.float32)
    w2s = np.ascontiguousarray(w2[idx])                  # [K,DF,D] f32
    b2s = np.ascontiguousarray(b2[idx], dtype=np.float32)

    # device tensors
    w1b = (S_W1 * w1s).astype(ml_dtypes.bfloat16)        # [K,D,DF]
    w2f = np.ascontiguousarray(
        _f8(S_W2 * w2s).reshape(K, FC, P, D).transpose(0, 2, 1, 3)
    )                                                    # [K,P,FC,D] fp8
    mB = (S_M * np.matmul(w1s, w2s)).astype(ml_dtypes.bfloat16)  # [K,D,D]

    # E-residual means (exact Gaussian statistics: u_f ~ N(0, ||w1[:,f]||)).
    mus = np.empty(K, np.float32)
    for k in range(K):
        sigma = np.linalg.norm(w1s[k], axis=0)           # [DF]
        mus[k] = _gelu_tanh_mean(sigma, b1s[k]).mean() - 0.0
    nmu = np.ascontiguousarray(
        np.broadcast_to(-mus[None, :], (P, K)), dtype=np.float32
    )

    gwsT = np.ascontiguousarray(
        gws.T.reshape(DC, P, K).transpose(1, 0, 2).reshape(P, DC * K)
    ).astype(ml_dtypes.bfloat16)
    gbb = np.ascontiguousarray(np.broadcast_to(gbs[None, :], (P, K)),
                               dtype=np.float32)
    b1p = np.ascontiguousarray(
        b1s.reshape(K, FC, P).transpose(0, 2, 1), dtype=np.float32
    )

    xf = x.reshape(Ttot, D)
    xT_bf = np.ascontiguousarray(xf.T).astype(ml_dtypes.bfloat16)

    nc = _get_nc(K, T, D, DF)
    in_maps = []
    for c in range(N_CORES):
        in_maps.append({
            "xT": np.ascontiguousarray(xT_bf[:, c * T:(c + 1) * T]),
            "w1b": w1b,
            "w2f": w2f,
            "mB": mB,
            "gwsT": gwsT,
            "gbb": gbb,
            "b1p": b1p,
            "nmu": nmu,
        })

    res = run_bass_kernel_spmd(
        nc, in_maps, list(range(N_CORES)), trace=trace, tmpdir=tmpdir
    )
    outp = np.concatenate(
        [res.results[c]["out"] for c in range(N_CORES)], axis=0
    ).reshape(B, S, D).astype(np.float32)

    # Exact rank-1 correction: sum_k gw_k * (mu_k*colsum(W2_k) + b2_k).
    # gw recomputed on host in fp32; mismatch vs device bf16 gw is O(1e-5).
    C = mus[:, None] * w2s.sum(axis=1) + b2s             # [K, D]
    logits = xf @ gws.T + gbs[None, :]
    gwh = _softmax(logits, axis=1).astype(np.float32)
    outp = outp + (gwh @ C).reshape(B, S, D)

    return outp, res


def kernel(**inputs) -> np.ndarray:
    outp, _ = run(inputs, trace=False)
    return outp


# revision 17
# speedup vs baseline: 1.2278x; 1.0009x over previous
"""Mixture-of-Experts Trainium2 kernel (8-core SPMD, token-sharded).

Reference: coarse top-K expert selection from the gate applied to the global
token sum, then dense K-expert FFN over all tokens with per-token softmax
gating over the K selected experts.

Hybrid-precision device strategy ("E-split"). The pointwise identity
    gelu(v) = v/2 + E(v),   E even, small (std ~0.1)
splits each expert's FFN into an exact linear path and a residual path:
    gelu(x@W1 + b1) @ W2 = x @ (W1@W2)/2 + (b1/2 + E(v)) @ W2
The linear path is a cheap [D,D] bf16 matmul (precomputed W1@W2 on host).
Only the residual E - mu (mean-removed, quantized fp8e4) goes through the
big [DF,D] contraction, which then runs as fp8 DoubleRow matmuls at ~1.8x
the bf16 rate.  L1 (x@W1, needed to evaluate E) stays bf16 for accuracy.
The exact rank-1 term sum_k gw_k * (mu_k*colsum(W2_k) + b2_k) is added on
host (gw recomputed exactly there; mismatch vs device gw is ~1e-5).

Scales: w1 x32 (psum=32u, gelu scale 1/32), E x4, w2 x32, lin M = 64*W1W2
=> combine psum = 128 * eo; softmax gating weights pre-divided by 128.

Per core (T=2048 tokens): gating bf16 (token-major), then per expert:
L1 64 psum tiles (4 bf16 MMs each) -> gelu (ACT) -> E fp8 (2 DVE ops),
combine 16 psum tiles (4 bf16 lin MMs + 8 fp8-DR MMs) -> acc (+)= po*gw.
Output stores stream per-tile during the last expert's combine.
"""

import numpy as np
import ml_dtypes
from contextlib import ExitStack

import bass_rust as _bass_rust
import concourse.bass as bass
import concourse.mybir as mybir
import concourse.tile as tile
from concourse.bass_utils import run_bass_kernel_spmd

BF16 = mybir.dt.bfloat16
FP8 = mybir.dt.float8e4
F32 = mybir.dt.float32
N_CORES = 8
P = 128

S_W1 = 0.5     # w1 pre-scale (L1 psum = u/2, so E = g - psum in ONE DVE op)
S_W2 = 128.0   # w2 pre-scale
S_PS = S_W2                # combine psum scale (=128)
S_M = S_PS / 2.0           # lin path: M = 64 * (W1 @ W2)


# ---------------------------------------------------------------------------
# Workaround for walrus "Too many sync wait commands": this walrus build
# accepts at most one semaphore wait in a single instruction's sync_info,
# but Tile's scheduler (and its kernel-tail drain) can attach several.
# Post-pass: move excess waits onto standalone EventSemaphore instructions
# inserted immediately before the offender on the same engine.
# ---------------------------------------------------------------------------
_split_ctr = [0]


def _split_multi_waits(nc):
    for f in nc.m.functions:
        for blk in f.blocks:
            insts = blk.instructions
            i = 0
            while i < len(insts):
                inst = insts[i]
                si = getattr(inst, "sync_info", None)
                waits = list(si.on_wait) if si is not None and si.on_wait else []
                if len(waits) > 1:
                    si.on_wait = waits[-1:]
                    for w in waits[:-1]:
                        _split_ctr[0] += 1
                        ev = mybir.InstEventSemaphore(
                            name=f"I-wsplit-{_split_ctr[0]}", ins=[], outs=[]
                        )
                        ev.engine = inst.engine
                        ev.sync_info = _bass_rust.SyncInfo(
                            on_wait=[w], on_update=[]
                        )
                        insts.insert(i, ev)
                        i += 1
                i += 1


# ---------------------------------------------------------------------------
# Device kernel
# ---------------------------------------------------------------------------
def build_moe_kernel(K: int, T: int, D: int, DF: int):
    assert T % 512 == 0 and D % P == 0 and DF % 256 == 0
    TT = T // P       # 128-token tiles
    TC = T // 512     # 512-token chunks
    DC = D // P       # D chunks of 128
    FC = DF // P      # F chunks of 128
    C8 = DF // 256    # DoubleRow chunks (256-deep each)

    nc = bass.Bass("TRN2", target_bir_lowering=False)

    xT = nc.declare_dram_parameter("xT", [D, T], BF16, isOutput=False)
    w1b = nc.declare_dram_parameter("w1b", [K, D, DF], BF16, isOutput=False)
    w2f = nc.declare_dram_parameter("w2f", [K, P, FC, D], FP8, isOutput=False)
    mB = nc.declare_dram_parameter("mB", [K, D, D], BF16, isOutput=False)
    gwsT = nc.declare_dram_parameter("gwsT", [P, DC * K], BF16, isOutput=False)
    gbb = nc.declare_dram_parameter("gbb", [P, K], F32, isOutput=False)
    b1p = nc.declare_dram_parameter("b1p", [K, P, FC], F32, isOutput=False)
    nmu = nc.declare_dram_parameter("nmu", [P, K], F32, isOutput=False)
    out = nc.declare_dram_parameter("out", [T, D], F32, isOutput=True)

    mult = mybir.AluOpType.mult
    add = mybir.AluOpType.add
    gelu_fn = mybir.ActivationFunctionType.Gelu_apprx_tanh
    exp_fn = mybir.ActivationFunctionType.Exp
    DR = mybir.MatmulPerfMode.DoubleRow

    with tile.TileContext(nc) as tc:
        with ExitStack() as ctx:
            persist = ctx.enter_context(tc.tile_pool(name="persist", bufs=1))
            w1p = ctx.enter_context(tc.tile_pool(name="w1p", bufs=2 * DC))
            w2p = ctx.enter_context(tc.tile_pool(name="w2p", bufs=2))
            ep = ctx.enter_context(tc.tile_pool(name="ep", bufs=C8 + 2))
            gp = ctx.enter_context(tc.tile_pool(name="gp", bufs=6))
            sm = ctx.enter_context(tc.tile_pool(name="sm", bufs=8))
            psA = ctx.enter_context(tc.tile_pool(name="psA", bufs=4, space="PSUM"))
            psB = ctx.enter_context(tc.tile_pool(name="psB", bufs=4, space="PSUM"))

            # ---- persistent loads. Emission order = DMA queue order:
            # tiny tensors first (gating needs gws immediately), then x
            # chunks interleaved with expert-0 w1 chunks so gating can start
            # on x chunk 0 while w1 still streams. ----
            gws_sb = persist.tile([P, DC * K], BF16, tag="gws", name="gws_sb")
            nc.sync.dma_start(gws_sb[:], gwsT[:])
            gbb_sb = persist.tile([P, K], F32, tag="gbb", name="gbb_sb")
            nc.sync.dma_start(gbb_sb[:], gbb[:])
            nmu_sb = persist.tile([P, K], F32, tag="nmu", name="nmu_sb")
            nc.sync.dma_start(nmu_sb[:], nmu[:])
            b1_sb = []
            for k in range(K):
                t = persist.tile([P, FC], F32, tag=f"b1_{k}", name=f"b1_{k}")
                nc.sync.dma_start(t[:], b1p[k])
                b1_sb.append(t)

            xt = [
                persist.tile([P, T], BF16, tag=f"xt{dc}", name=f"xt{dc}")
                for dc in range(DC)
            ]
            w1t0 = [
                w1p.tile([P, DF], BF16, tag="w1", name=f"w1_0_{dc}")
                for dc in range(DC)
            ]
            for dc in range(DC):
                nc.sync.dma_start(xt[dc][:], xT[dc * P:(dc + 1) * P, :])
            for dc in range(DC):
                nc.sync.dma_start(w1t0[dc][:], w1b[0, dc * P:(dc + 1) * P, :])
            w2t0 = w2p.tile([P, FC, D], FP8, tag="w2", name="w2_0")
            nc.sync.dma_start(w2t0[:], w2f[0])

            # lin-path moving matrices, all experts resident
            m_sb = []
            for k in range(K):
                t = persist.tile([P, DC, D], BF16, tag=f"m{k}", name=f"m{k}")
                for dc in range(DC):
                    nc.sync.dma_start(
                        t[:, dc:dc + 1, :], mB[k, dc * P:(dc + 1) * P, :]
                    )
                m_sb.append(t)

            acc = [
                persist.tile([P, D], F32, tag=f"acc{t}", name=f"acc{t}")
                for t in range(TT)
            ]
            gw_sb = [
                persist.tile([P, K], F32, tag=f"gw{t}", name=f"gw{t}")
                for t in range(TT)
            ]

            # ---- gating softmax (token-major); gw_sb holds softmax/S_PS ----
            # dc-outer rounds of 4 token tiles: the first matmuls only need
            # x chunk 0, so gating starts while the rest of x still streams.
            RT = 4
            for rnd in range(0, TT, RT):
                tts = range(rnd, min(rnd + RT, TT))
                pls = {}
                for tt in tts:
                    pls[tt] = psB.tile([P, 512], F32, tag="po", name=f"pl{tt}")
                for dc in range(DC):
                    for tt in tts:
                        nc.tensor.matmul(
                            pls[tt][:, 0:K],
                            xt[dc][:, tt * P:(tt + 1) * P],
                            gws_sb[:, dc * K:(dc + 1) * K],
                            start=(dc == 0),
                            stop=(dc == DC - 1),
                        )
                for tt in tts:
                    pl = pls[tt]
                    l_sb = sm.tile([P, K], F32, tag="l", name=f"l{tt}")
                    nc.vector.tensor_add(l_sb[:], pl[:, 0:K], gbb_sb[:])
                    negmax = sm.tile([P, 1], F32, tag="negmax",
                                     name=f"negmax{tt}")
                    nc.vector.reduce_max(
                        negmax[:], l_sb[:], axis=mybir.AxisListType.X,
                        negate=True,
                    )
                    z = sm.tile([P, 1], F32, tag="z", name=f"z{tt}")
                    nc.scalar.activation(
                        gw_sb[tt][:], l_sb[:], exp_fn,
                        bias=negmax[:, 0:1], accum_out=z[:, 0:1],
                    )
                    rz = sm.tile([P, 1], F32, tag="rz", name=f"rz{tt}")
                    nc.vector.reciprocal(rz[:], z[:, 0:1])
                    nc.vector.tensor_scalar(
                        gw_sb[tt][:], gw_sb[tt][:], rz[:, 0:1], 1.0 / S_PS,
                        op0=mult, op1=mult,
                    )

            # ---- experts ----
            for k in range(K):
                if k == 0:
                    w1t, w2t8 = w1t0, w2t0
                else:
                    w1t = []
                    for dc in range(DC):
                        t = w1p.tile([P, DF], BF16, tag="w1", name=f"w1_{k}_{dc}")
                        nc.sync.dma_start(t[:], w1b[k, dc * P:(dc + 1) * P, :])
                        w1t.append(t)
                    w2t8 = w2p.tile([P, FC, D], FP8, tag="w2", name=f"w2_{k}")
                    nc.sync.dma_start(w2t8[:], w2f[k])

                # L1 (bf16): psum = u/2  [f-major], then
                #   g = gelu(2*psum + b1)            (ACT, bf16)
                #   E8 = (g + (-mu)) - psum           (DVE stt, fp8) = E - mu
                ec = [
                    ep.tile([P, 2, T], FP8, tag="E", name=f"E_{k}_{c}")
                    for c in range(C8)
                ]
                for fc in range(FC):
                    for tcc in range(TC):
                        ph = psA.tile([P, 512], F32, tag="ph",
                                      name=f"ph_{k}_{fc}_{tcc}")
                        for dc in range(DC):
                            nc.tensor.matmul(
                                ph[:],
                                w1t[dc][:, fc * P:(fc + 1) * P],
                                xt[dc][:, tcc * 512:(tcc + 1) * 512],
                                start=(dc == 0),
                                stop=(dc == DC - 1),
                            )
                        g = gp.tile([P, 512], BF16, tag="g",
                                    name=f"g_{k}_{fc}_{tcc}")
                        nc.scalar.activation(
                            g[:], ph[:], gelu_fn,
                            bias=b1_sb[k][:, fc:fc + 1], scale=1.0 / S_W1,
                        )
                        nc.vector.scalar_tensor_tensor(
                            ec[fc // 2][:, (fc % 2):(fc % 2) + 1,
                                        tcc * 512:(tcc + 1) * 512],
                            g[:], nmu_sb[:, k:k + 1], ph[:],
                            op0=add, op1=mybir.AluOpType.subtract,
                        )

                # combine: po = x@M (bf16) + E8 @ w2 (fp8 DoubleRow)
                #          acc (+)= po * gw
                for tt in range(TT):
                    po = psB.tile([P, 512], F32, tag="po", name=f"po_{k}_{tt}")
                    for dc in range(DC):
                        nc.tensor.matmul(
                            po[:, 0:D],
                            xt[dc][:, tt * P:(tt + 1) * P],
                            m_sb[k][:, dc:dc + 1, :],
                            start=(dc == 0),
                            stop=False,
                        )
                    for c in range(C8):
                        nc.tensor.matmul(
                            po[:, 0:D],
                            ec[c][:, :, tt * P:(tt + 1) * P],
                            w2t8[:, 2 * c:2 * c + 2, :],
                            start=False,
                            stop=(c == C8 - 1),
                            perf_mode=DR,
                        )
                    if k == 0:
                        nc.vector.tensor_scalar_mul(
                            acc[tt][:], po[:, 0:D], gw_sb[tt][:, 0:1]
                        )
                    else:
                        nc.vector.scalar_tensor_tensor(
                            acc[tt][:], po[:, 0:D], gw_sb[tt][:, k:k + 1],
                            acc[tt][:], op0=mult, op1=add,
                        )
                    if k == K - 1:
                        nc.sync.dma_start(
                            out[tt * P:(tt + 1) * P, :], acc[tt][:]
                        )

    _split_multi_waits(nc)
    return nc


# ---------------------------------------------------------------------------
# Host wrapper
# ---------------------------------------------------------------------------
_NC_CACHE: dict = {}


def _get_nc(K: int, T: int, D: int, DF: int):
    key = (K, T, D, DF)
    if key not in _NC_CACHE:
        _NC_CACHE[key] = build_moe_kernel(K, T, D, DF)
    return _NC_CACHE[key]


def _softmax(x, axis=-1):
    m = np.max(x, axis=axis, keepdims=True)
    e = np.exp(x - m)
    return e / np.sum(e, axis=axis, keepdims=True)


def _gelu_tanh_mean(sigma, b1):
    """E_z[gelu_tanh(b1 + sigma*z)], z~N(0,1), vectorized over features."""
    zg = np.linspace(-8.0, 8.0, 401)
    wg = np.exp(-0.5 * zg * zg)
    wg /= wg.sum()
    v = b1[:, None] + sigma[:, None] * zg[None, :]       # [DF, NZ]
    g = 0.5 * v * (1.0 + np.tanh(np.sqrt(2.0 / np.pi) * (v + 0.044715 * v**3)))
    return (g * wg[None, :]).sum(axis=1)                 # [DF]


def _f8(a):
    return np.clip(np.asarray(a, np.float32), -240.0, 240.0).astype(
        ml_dtypes.float8_e4m3fn
    )


def run(inputs: dict, trace: bool = False, tmpdir: str | None = None):
    x = np.asarray(inputs["x"], dtype=np.float32)
    gate_w = np.asarray(inputs["gate_w"], dtype=np.float32)
    gate_b = np.asarray(inputs["gate_b"], dtype=np.float32)
    w1 = np.asarray(inputs["w1"], dtype=np.float32)
    b1 = np.asarray(inputs["b1"], dtype=np.float32)
    w2 = np.asarray(inputs["w2"], dtype=np.float32)
    b2 = np.asarray(inputs["b2"], dtype=np.float32)
    K = int(inputs["num_available"])

    B, S, D = x.shape
    DF = w1.shape[2]
    Ttot = B * S
    T = Ttot // N_CORES
    DC = D // P
    FC = DF // P

    # Coarse routing on host (tiny): gate applied to the global token sum.
    ksum = x.sum(axis=(0, 1))
    coarse = gate_w @ ksum + gate_b
    idx = np.argsort(-coarse, kind="stable")[:K]

    gws, gbs = gate_w[idx], gate_b[idx]
    w1s = np.ascontiguousarray(w1[idx])                  # [K,D,DF] f32
    b1s = np.ascontiguousarray(b1[idx], dtype=np.float32)
    w2s = np.ascontiguousarray(w2[idx])                  # [K,DF,D] f32
    b2s = np.ascontiguousarray(b2[idx], dtype=np.float32)

    # device tensors
    w1b = (S_W1 * w1s).astype(ml_dtypes.bfloat16)        # [K,D,DF]
    w2f = np.ascontiguousarray(
        _f8(S_W2 * w2s).reshape(K, FC, P, D).transpose(0, 2, 1, 3)
    )                                                    # [K,P,FC,D] fp8
    mB = (S_M * np.matmul(w1s, w2s)).astype(ml_dtypes.bfloat16)  # [K,D,D]

    # E-residual means (exact Gaussian statistics: u_f ~ N(0, ||w1[:,f]||)).
    mus = np.empty(K, np.float32)
    for k in range(K):
        sigma = np.linalg.norm(w1s[k], axis=0)           # [DF]
        mus[k] = _gelu_tanh_mean(sigma, b1s[k]).mean() - 0.0
    nmu = np.ascontiguousarray(
        np.broadcast_to(-mus[None, :], (P, K)), dtype=np.float32
    )

    gwsT = np.ascontiguousarray(
        gws.T.reshape(DC, P, K).transpose(1, 0, 2).reshape(P, DC * K)
    ).astype(ml_dtypes.bfloat16)
    gbb = np.ascontiguousarray(np.broadcast_to(gbs[None, :], (P, K)),
                               dtype=np.float32)
    b1p = np.ascontiguousarray(
        b1s.reshape(K, FC, P).transpose(0, 2, 1), dtype=np.float32
    )

    xf = x.reshape(Ttot, D)
    xT_bf = np.ascontiguousarray(xf.T).astype(ml_dtypes.bfloat16)

    nc = _get_nc(K, T, D, DF)
    in_maps = []
    for c in range(N_CORES):
        in_maps.append({
            "xT": np.ascontiguousarray(xT_bf[:, c * T:(c + 1) * T]),
            "w1b": w1b,
            "w2f": w2f,
            "mB": mB,
            "gwsT": gwsT,
            "gbb": gbb,
            "b1p": b1p,
            "nmu": nmu,
        })

    res = run_bass_kernel_spmd(
        nc, in_maps, list(range(N_CORES)), trace=trace, tmpdir=tmpdir
    )
    outp = np.concatenate(
        [res.results[c]["out"] for c in range(N_CORES)], axis=0
    ).reshape(B, S, D).astype(np.float32)

    # Exact rank-1 correction: sum_k gw_k * (mu_k*colsum(W2_k) + b2_k).
    # gw recomputed on host in fp32; mismatch vs device bf16 gw is O(1e-5).
    C = mus[:, None] * w2s.sum(axis=1) + b2s             # [K, D]
    logits = xf @ gws.T + gbs[None, :]
    gwh = _softmax(logits, axis=1).astype(np.float32)
    outp = outp + (gwh @ C).reshape(B, S, D)

    return outp, res


def kernel(**inputs) -> np.ndarray:
    outp, _ = run(inputs, trace=False)
    return outp
